# revision 10
# baseline (speedup 1.0000x reference)
"""Multi-head self-attention (B=2, S=2048, D=1024, H=16) on 8 TRN2 NeuronCores.

Sharding: core c handles batch b = c//4 and head group g = c%4 (4 heads each).
Each core computes qkv projection for its heads, masked-softmax attention, and
a partial output projection; the host sums the 4 partial outputs per batch.

Fast path (causal mask): scores are computed transposed (keys on partitions,
queries on the free dim) so the P^T tile the PV matmul needs comes straight
out of the exp() with no transpose. Diagonal 128x512 score blocks are trimmed
to their causally-valid column range; the only masked region left is the
128x128 triangle at the start of each diagonal block, handled by one shared
triu(ones) multiply. Softmax skips max-subtraction (scores are O(1)); the
denominator falls out of a ones-column appended to the V stationary. The
q/k/v projections for strips 1-3 and v tiles 4-15 are emitted as PE "filler"
between attention iterations so the tensor engine never idles while exp runs.
All cross-phase producer/consumer pairs use whole tiles (per-strip q/k, per
key-tile v, per-strip attention outputs) so dependency tracking is exact.
"""

from collections import deque
from contextlib import ExitStack

import numpy as np

import concourse.bass as bass
import concourse.tile as tile
from concourse import bacc, mybir
from concourse.bass_utils import run_bass_kernel_spmd

F32 = mybir.dt.float32
F16 = mybir.dt.float16

B, S, D, H, DH = 2, 2048, 1024, 16, 64
HPC = 4          # heads per core
NCORES = 8
KT = S // 128    # 16 key tiles of 128
QS = S // 512    # 4 query strips of 512
DKT = D // 128   # 8 contraction tiles for the projections


def _to_f16(x):
    return np.ascontiguousarray(x).astype(np.float16)


def _build_causal():
    """Specialized build for the exact causal (tril) mask."""
    nc = bacc.Bacc()

    xT0 = nc.dram_tensor("xT0", [D, 512], F16, kind="ExternalInput")
    xTr = nc.dram_tensor("xTr", [D, S - 512], F16, kind="ExternalInput")
    wqk = nc.dram_tensor("wqk", [D, 512], F16, kind="ExternalInput")
    wv = nc.dram_tensor("wv", [D, 256], F16, kind="ExternalInput")
    wo = nc.dram_tensor("wo", [256, D], F16, kind="ExternalInput")
    trimask = nc.dram_tensor("trimask", [128, 128], F16, kind="ExternalInput")
    vones = nc.dram_tensor("vones", [128, KT * HPC], F16, kind="ExternalInput")
    out = nc.dram_tensor("out", [S, D], F16, kind="ExternalOutput")

    with tile.TileContext(nc) as tc, ExitStack() as top:
        persist = top.enter_context(tc.tile_pool(name="persist", bufs=1))

        # qks[ct][ss]: transposed projections [proj-col, token] per 512-token
        # strip; ct 0-1 = q heads (0,1),(2,3) scaled by 1/sqrt(dh) host-side;
        # ct 2-3 = k heads.
        qks = [
            [
                persist.tile([128, 512], F16, name=f"qk{ct}_{ss}", tag=f"qk{ct}_{ss}")
                for ss in range(QS)
            ]
            for ct in range(4)
        ]
        # vx[kt]: per key-tile kt and head h, [128 tokens, 64 dims + ones col]
        # at column offset 65*h.
        vx = [
            persist.tile([128, HPC * 65], F16, name=f"vx{kt}", tag=f"vx{kt}")
            for kt in range(KT)
        ]
        # attention outputs per strip; ots[t][qs] holds heads 2t,2t+1
        ots = [
            [
                persist.tile([128, 512], F16, name=f"ot{t}_{qs}", tag=f"ot{t}_{qs}")
                for qs in range(QS)
            ]
            for t in range(2)
        ]
        wo_t = [persist.tile([128, D], F16, name=f"wo{t}", tag=f"wo{t}") for t in range(2)]
        tri = persist.tile([128, 128], F16, tag="tri")

        xt0 = [persist.tile([128, 512], F16, name=f"xt0_{kt}", tag=f"xt0_{kt}") for kt in range(DKT)]
        xtr = [persist.tile([128, S - 512], F16, name=f"xtr{kt}", tag=f"xtr{kt}") for kt in range(DKT)]
        wqk_t = [persist.tile([128, 512], F16, name=f"wqk{kt}", tag=f"wqk{kt}") for kt in range(DKT)]
        wv_t = [persist.tile([128, 256], F16, name=f"wv{kt}", tag=f"wv{kt}") for kt in range(DKT)]

        # the very first matmul needs only these two tiles: put them at the
        # head of two separate rings so they land first
        nc.sync.dma_start(xt0[0][:], xT0[0:128, :])
        nc.scalar.dma_start(wqk_t[0][:], wqk[0:128, :])
        # everything else round-robins over three rings in need-order
        rings = [nc.sync, nc.scalar, nc.gpsimd]
        rr = 0

        def dma(dst, src):
            nonlocal rr
            rings[rr % 3].dma_start(dst, src)
            rr += 1

        for kt in range(1, DKT):
            dma(wqk_t[kt][:], wqk[128 * kt : 128 * kt + 128, :])
            dma(xt0[kt][:], xT0[128 * kt : 128 * kt + 128, :])
        for kt in range(DKT):
            dma(wv_t[kt][:], wv[128 * kt : 128 * kt + 128, :])
        for t in range(2):
            dma(wo_t[t][:], wo[128 * t : 128 * t + 128, :])
        dma(tri[:], trimask[:])
        for kt in range(KT):
            dma(
                vx[kt][:].rearrange("p (h c) -> p h c", c=65)[:, :, 64:65],
                vones[:, HPC * kt : HPC * kt + HPC].rearrange(
                    "p (h o) -> p h o", o=1
                ),
            )
        for kt in range(DKT):
            dma(xtr[kt][:], xTr[128 * kt : 128 * kt + 128, :])

        def xslice(kt, lo, hi):
            # columns [lo, hi) of the logical xT tile kt
            if hi <= 512:
                return xt0[kt][:, lo:hi]
            return xtr[kt][:, lo - 512 : hi - 512]

        # ---- phase 1a: q/k strip 0 + v tiles 0-3 ----
        with ExitStack() as pha:
            psA = pha.enter_context(tc.tile_pool(name="psA", bufs=1, space="PSUM"))
            psV = pha.enter_context(tc.tile_pool(name="psV", bufs=2, space="PSUM"))
            pa = [
                psA.tile([128, 512], F32, name=f"pa{ct}", tag=f"pa{ct}")
                for ct in range(4)
            ]
            # kt-outer so the first matmul depends only on the kt=0 tiles
            for kt in range(DKT):
                for ct in range(4):
                    nc.tensor.matmul(
                        pa[ct][:],
                        wqk_t[kt][:, 128 * ct : 128 * ct + 128],
                        xt0[kt][:],
                        start=(kt == 0),
                        stop=(kt == DKT - 1),
                    )
            for ct in range(4):
                nc.vector.tensor_copy(qks[ct][0][:], pa[ct][:])

            for st in range(4):
                ps = psV.tile([128, 256], F32, tag="psv")
                for kt in range(DKT):
                    nc.tensor.matmul(
                        ps[:],
                        xslice(kt, 128 * st, 128 * st + 128),
                        wv_t[kt][:],
                        start=(kt == 0),
                        stop=(kt == DKT - 1),
                    )
                dst = vx[st][:].rearrange("p (h c) -> p h c", c=65)[:, :, 0:64]
                nc.vector.tensor_copy(dst, ps[:].rearrange("p (h c) -> p h c", c=64))

        # ---- phase 2: attention, with remaining projections as PE filler ----
        with ExitStack() as ph2:
            osb = ph2.enter_context(tc.tile_pool(name="osb", bufs=3))
            ptp = ph2.enter_context(tc.tile_pool(name="pt", bufs=4))
            nrm = ph2.enter_context(tc.tile_pool(name="nrm", bufs=3))
            ps_st = ph2.enter_context(
                tc.tile_pool(name="ps_st", bufs=2, space="PSUM")
            )
            ps_o = ph2.enter_context(tc.tile_pool(name="ps_o", bufs=2, space="PSUM"))
            ups = ph2.enter_context(tc.tile_pool(name="ups", bufs=2, space="PSUM"))

            def emit_qk_strip(ct, ss):
                ps = ups.tile([128, 512], F32, name="ps", tag="upsqk")
                for kt in range(DKT):
                    nc.tensor.matmul(
                        ps[:],
                        wqk_t[kt][:, 128 * ct : 128 * ct + 128],
                        xtr[kt][:, 512 * ss - 512 : 512 * ss],
                        start=(kt == 0),
                        stop=(kt == DKT - 1),
                    )
                nc.vector.tensor_copy(qks[ct][ss][:], ps[:])

            def emit_v(st):
                psf = ups.tile([128, 512], F32, name="psf", tag="upsqk")
                ps = psf[:, 0:256]
                for kt in range(DKT):
                    nc.tensor.matmul(
                        ps,
                        xslice(kt, 128 * st, 128 * st + 128),
                        wv_t[kt][:],
                        start=(kt == 0),
                        stop=(kt == DKT - 1),
                    )
                dst = vx[st][:].rearrange("p (h c) -> p h c", c=65)[:, :, 0:64]
                nc.vector.tensor_copy(dst, ps.rearrange("p (h c) -> p h c", c=64))

            # filler: strip ss=qs+1 and v tiles 4qs+4..4qs+7 are consumed
            # during strip qs (2 units after each of the 4 head iterations)
            fillers = deque()
            for ss in range(1, 4):
                for j in range(4):
                    fillers.append(lambda ct=j, s=ss: emit_qk_strip(ct, s))
                    fillers.append(lambda st=4 * ss + j: emit_v(st))
            if True:  # bisect: no interleave
                while fillers:
                    fillers.popleft()()

            for qs in range(QS):
                for h in range(HPC):
                    hh = 64 * (h % 2)
                    qT = qks[h // 2][qs][hh : hh + 64, :]

                    # block list: (ki, width, q-col offset within strip)
                    blocks = [(ki, 512, 0) for ki in range(4 * qs)]
                    blocks += [
                        (4 * qs + st, 512 - 128 * st, 128 * st) for st in range(4)
                    ]
                    chunks = [blocks[j : j + 2] for j in range(0, len(blocks), 2)]

                    pts = []
                    for chunk in chunks:
                        w = sum(c[1] for c in chunk)
                        pst = ps_st.tile([128, 1024], F32, tag="pst")
                        off = 0
                        offs = []
                        for ki, cw, qo in chunk:
                            kk = qks[2 + h // 2][ki // 4]
                            nc.tensor.matmul(
                                pst[:, off : off + cw],
                                kk[hh : hh + 64, 128 * (ki % 4) : 128 * (ki % 4) + 128],
                                qT[:, qo : qo + cw],
                                start=True,
                                stop=True,
                            )
                            offs.append(off)
                            off += cw
                        pt = ptp.tile([128, 1024], F16, tag="pt")
                        nc.scalar.activation(
                            pt[:, :w], pst[:, :w], mybir.ActivationFunctionType.Exp
                        )
                        # triangle mask on the first 128 cols of diagonal blocks
                        for (ki, cw, qo), off in zip(chunk, offs):
                            if ki >= 4 * qs:
                                nc.vector.tensor_mul(
                                    pt[:, off : off + 128],
                                    pt[:, off : off + 128],
                                    tri[:],
                                )
                        pts.append((chunk, offs, pt))

                    if qs < 3 and fillers:
                        fillers.popleft()()
                        fillers.popleft()()

                    po = ps_o.tile([65, 512], F32, tag="po")
                    nmm = sum(len(c) for c, _, _ in pts)
                    done = 0
                    for chunk, offs, pt in pts:
                        for (ki, cw, qo), off in zip(chunk, offs):
                            nc.tensor.matmul(
                                po[:, qo : qo + cw],
                                vx[ki][:, 65 * h : 65 * h + 65],
                                pt[:, off : off + cw],
                                start=(done == 0),
                                stop=(done == nmm - 1),
                            )
                            done += 1

                    # normalize: row 64 of po is the softmax denominator
                    rden = nrm.tile([1, 512], F32, tag="rden")
                    nc.vector.tensor_copy(rden[:], po[64:65, :])
                    rrec = nrm.tile([1, 512], F32, tag="rrec")
                    nc.vector.reciprocal_approx_fast(rrec[:], rden[:])
                    rb = nrm.tile([64, 512], F32, tag="rb")
                    nc.gpsimd.partition_broadcast(rb[:], rrec[:])
                    nc.vector.tensor_mul(
                        ots[h // 2][qs][hh : hh + 64, :],
                        po[0:64, :],
                        rb[:],
                    )

                # output projection for this strip's 4 token tiles
                for st4 in range(4):
                    for oc in range(2):
                        pop = ups.tile([128, 512], F32, name="pop", tag="upsqk")
                        for t in range(2):
                            nc.tensor.matmul(
                                pop[:],
                                ots[t][qs][:, 128 * st4 : 128 * st4 + 128],
                                wo_t[t][:, 512 * oc : 512 * oc + 512],
                                start=(t == 0),
                                stop=(t == 1),
                            )
                        ob = osb.tile([128, 512], F16, tag="ob")
                        nc.vector.tensor_copy(ob[:], pop[:])
                        nc.sync.dma_start(
                            out[
                                128 * (4 * qs + st4) : 128 * (4 * qs + st4) + 128,
                                512 * oc : 512 * oc + 512,
                            ],
                            ob[:],
                        )

    nc.finalize()
    return nc


_cache = {}


def _get_nc():
    if "causal" not in _cache:
        _cache["causal"] = _build_causal()
    return _cache["causal"]


def _check_causal(mask):
    tril = np.tril(np.ones((S, S), dtype=mask.dtype))
    return all(np.array_equal(np.asarray(mask[b]), tril) for b in range(B))


def _prepare(x, mask, w_qkv, w_out):
    """Host-side sharding. Returns in_maps (one per core)."""
    scale = 1.0 / np.sqrt(DH)

    tri = np.triu(np.ones((128, 128), np.float16))  # keep iff qcol >= krow

    in_maps = []
    for c in range(NCORES):
        b, g = c // 4, c % 4
        heads = range(4 * g, 4 * g + 4)
        xTb = _to_f16(np.ascontiguousarray(x[b].T))
        wq = np.concatenate(
            [w_qkv[:, 64 * h : 64 * h + 64] for h in heads], axis=1
        ) * scale
        wk = np.concatenate(
            [w_qkv[:, D + 64 * h : D + 64 * h + 64] for h in heads], axis=1
        )
        wvv = np.concatenate(
            [w_qkv[:, 2 * D + 64 * h : 2 * D + 64 * h + 64] for h in heads], axis=1
        )
        woo = np.concatenate(
            [w_out[64 * h : 64 * h + 64, :] for h in heads], axis=0
        )
        in_maps.append(
            {
                "xT0": np.ascontiguousarray(xTb[:, 0:512]),
                "xTr": np.ascontiguousarray(xTb[:, 512:]),
                "wqk": _to_f16(np.concatenate([wq, wk], axis=1)),
                "wv": _to_f16(wvv),
                "wo": _to_f16(np.ascontiguousarray(woo)),
                "trimask": tri,
                "vones": np.ones((128, KT * HPC), np.float16),
            }
        )
    return in_maps


def _run(x, mask, w_qkv, w_out, trace=False, trace_cores=None):
    assert _check_causal(mask), "kernel specialized for the causal (tril) mask"
    in_maps = _prepare(x, mask, w_qkv, w_out)
    nc = _get_nc()
    res = run_bass_kernel_spmd(
        nc,
        in_maps,
        core_ids=list(range(NCORES)),
        trace=trace,
        trace_cores=trace_cores,
    )
    outs = np.stack(
        [
            sum(
                res.results[4 * b + g]["out"].astype(np.float32) for g in range(4)
            )
            for b in range(B)
        ]
    )
    return outs.astype(np.float32), res


def kernel(x, mask, w_qkv, w_out):
    x = np.asarray(x, np.float32)
    mask = np.asarray(mask)
    w_qkv = np.asarray(w_qkv, np.float32)
    w_out = np.asarray(w_out, np.float32)
    out, _ = _run(x, mask, w_qkv, w_out)
    return out


# revision 19
# speedup vs baseline: 1.0011x; 1.0011x over previous
"""Multi-head self-attention (B=2, S=2048, D=1024, H=16) on 8 TRN2 NeuronCores.

Sharding: core c handles batch b = c//4 and head group g = c%4 (4 heads each).
Each core computes qkv projection for its heads, masked-softmax attention, and
a partial output projection; the host sums the 4 partial outputs per batch.

Fast path (causal mask): scores are computed transposed (keys on partitions,
queries on the free dim) so the P^T tile the PV matmul needs comes straight
out of the exp() with no transpose. Diagonal 128x512 score blocks are trimmed
to their causally-valid column range; the only masked region left is the
128x128 triangle at the start of each diagonal block, handled by one shared
triu(ones) multiply. Softmax skips max-subtraction (scores are O(1)); the
denominator falls out of a ones-column appended to the V stationary. The
q/k/v projections for strips 1-3 and v tiles 4-15 are emitted as PE "filler"
between attention iterations so the tensor engine never idles while exp runs.
All cross-phase producer/consumer pairs use whole tiles (per-strip q/k, per
key-tile v, per-strip attention outputs) so dependency tracking is exact.
"""

from collections import deque
from contextlib import ExitStack

import numpy as np

import concourse.bass as bass
import concourse.tile as tile
from concourse import bacc, mybir
from concourse.bass_utils import run_bass_kernel_spmd

F32 = mybir.dt.float32
F16 = mybir.dt.float16

B, S, D, H, DH = 2, 2048, 1024, 16, 64
HPC = 4          # heads per core
NCORES = 8
KT = S // 128    # 16 key tiles of 128
QS = S // 512    # 4 query strips of 512
DKT = D // 128   # 8 contraction tiles for the projections


def _to_f16(x):
    return np.ascontiguousarray(x).astype(np.float16)


def _build_causal():
    """Specialized build for the exact causal (tril) mask."""
    nc = bacc.Bacc()

    xT0 = nc.dram_tensor("xT0", [D, 512], F16, kind="ExternalInput")
    xTr = nc.dram_tensor("xTr", [D, S - 512], F16, kind="ExternalInput")
    wqk = nc.dram_tensor("wqk", [D, 512], F16, kind="ExternalInput")
    wv = nc.dram_tensor("wv", [D, 256], F16, kind="ExternalInput")
    wo = nc.dram_tensor("wo", [256, D], F16, kind="ExternalInput")
    trimask = nc.dram_tensor("trimask", [128, 128], F16, kind="ExternalInput")
    vones = nc.dram_tensor("vones", [128, KT * HPC], F16, kind="ExternalInput")
    out = nc.dram_tensor("out", [S, D], F16, kind="ExternalOutput")

    with tile.TileContext(nc) as tc, ExitStack() as top:
        persist = top.enter_context(tc.tile_pool(name="persist", bufs=1))

        # qks[ct][ss]: transposed projections [proj-col, token] per 512-token
        # strip; ct 0-1 = q heads (0,1),(2,3) scaled by 1/sqrt(dh) host-side;
        # ct 2-3 = k heads.
        qks = [
            [
                persist.tile([128, 512], F16, name=f"qk{ct}_{ss}", tag=f"qk{ct}_{ss}")
                for ss in range(QS)
            ]
            for ct in range(4)
        ]
        # vx[kt]: per key-tile kt and head h, [128 tokens, 64 dims + ones col]
        # at column offset 65*h.
        vx = [
            persist.tile([128, HPC * 65], F16, name=f"vx{kt}", tag=f"vx{kt}")
            for kt in range(KT)
        ]
        # attention outputs per strip; ots[t][qs] holds heads 2t,2t+1
        ots = [
            [
                persist.tile([128, 512], F16, name=f"ot{t}_{qs}", tag=f"ot{t}_{qs}")
                for qs in range(QS)
            ]
            for t in range(2)
        ]
        wo_t = [persist.tile([128, D], F16, name=f"wo{t}", tag=f"wo{t}") for t in range(2)]
        tri = persist.tile([128, 128], F16, tag="tri")

        xt0 = [persist.tile([128, 512], F16, name=f"xt0_{kt}", tag=f"xt0_{kt}") for kt in range(DKT)]
        xtr = [persist.tile([128, S - 512], F16, name=f"xtr{kt}", tag=f"xtr{kt}") for kt in range(DKT)]
        wqk_t = [persist.tile([128, 512], F16, name=f"wqk{kt}", tag=f"wqk{kt}") for kt in range(DKT)]
        wv_t = [persist.tile([128, 256], F16, name=f"wv{kt}", tag=f"wv{kt}") for kt in range(DKT)]

        # the very first matmul needs only these two tiles: put them at the
        # head of two separate rings so they land first
        nc.sync.dma_start(xt0[0][:], xT0[0:128, :])
        nc.scalar.dma_start(wqk_t[0][:], wqk[0:128, :])
        # everything else round-robins over three rings in need-order
        rings = [nc.sync, nc.scalar, nc.gpsimd]
        rr = 0

        def dma(dst, src):
            nonlocal rr
            rings[rr % 3].dma_start(dst, src)
            rr += 1

        for kt in range(1, DKT):
            dma(wqk_t[kt][:], wqk[128 * kt : 128 * kt + 128, :])
            dma(xt0[kt][:], xT0[128 * kt : 128 * kt + 128, :])
        for kt in range(DKT):
            dma(wv_t[kt][:], wv[128 * kt : 128 * kt + 128, :])
        for t in range(2):
            dma(wo_t[t][:], wo[128 * t : 128 * t + 128, :])
        dma(tri[:], trimask[:])
        for kt in range(KT):
            dma(
                vx[kt][:].rearrange("p (h c) -> p h c", c=65)[:, :, 64:65],
                vones[:, HPC * kt : HPC * kt + HPC].rearrange(
                    "p (h o) -> p h o", o=1
                ),
            )
        for kt in range(DKT):
            dma(xtr[kt][:], xTr[128 * kt : 128 * kt + 128, :])

        def xslice(kt, lo, hi):
            # columns [lo, hi) of the logical xT tile kt
            if hi <= 512:
                return xt0[kt][:, lo:hi]
            return xtr[kt][:, lo - 512 : hi - 512]

        # ---- phase 1a: q/k strip 0 + v tiles 0-3 ----
        with ExitStack() as pha:
            psA = pha.enter_context(tc.tile_pool(name="psA", bufs=1, space="PSUM"))
            psV = pha.enter_context(tc.tile_pool(name="psV", bufs=2, space="PSUM"))
            pa = [
                psA.tile([128, 512], F32, name=f"pa{ct}", tag=f"pa{ct}")
                for ct in range(4)
            ]
            # kt-outer so the first matmul depends only on the kt=0 tiles
            for kt in range(DKT):
                for ct in range(4):
                    nc.tensor.matmul(
                        pa[ct][:],
                        wqk_t[kt][:, 128 * ct : 128 * ct + 128],
                        xt0[kt][:],
                        start=(kt == 0),
                        stop=(kt == DKT - 1),
                    )
            for ct in range(4):
                nc.vector.tensor_copy(qks[ct][0][:], pa[ct][:])

            for st in range(4):
                ps = psV.tile([128, 256], F32, tag="psv")
                for kt in range(DKT):
                    nc.tensor.matmul(
                        ps[:],
                        xslice(kt, 128 * st, 128 * st + 128),
                        wv_t[kt][:],
                        start=(kt == 0),
                        stop=(kt == DKT - 1),
                    )
                dst = vx[st][:].rearrange("p (h c) -> p h c", c=65)[:, :, 0:64]
                nc.vector.tensor_copy(dst, ps[:].rearrange("p (h c) -> p h c", c=64))

        # ---- phase 2: attention, with remaining projections as PE filler ----
        with ExitStack() as ph2:
            osb = ph2.enter_context(tc.tile_pool(name="osb", bufs=3))
            ptp = ph2.enter_context(tc.tile_pool(name="pt", bufs=4))
            nrm = ph2.enter_context(tc.tile_pool(name="nrm", bufs=3))
            ps_st = ph2.enter_context(
                tc.tile_pool(name="ps_st", bufs=2, space="PSUM")
            )
            ps_o = ph2.enter_context(tc.tile_pool(name="ps_o", bufs=2, space="PSUM"))
            ups = ph2.enter_context(tc.tile_pool(name="ups", bufs=1, space="PSUM"))
            fil = ph2.enter_context(tc.tile_pool(name="fil", bufs=1, space="PSUM"))

            def emit_qk_strip(ct, ss):
                ps = fil.tile([128, 512], F32, name="ps", tag="fps")
                for kt in range(DKT):
                    nc.tensor.matmul(
                        ps[:],
                        wqk_t[kt][:, 128 * ct : 128 * ct + 128],
                        xtr[kt][:, 512 * ss - 512 : 512 * ss],
                        start=(kt == 0),
                        stop=(kt == DKT - 1),
                    )
                nc.vector.tensor_copy(qks[ct][ss][:], ps[:])

            def emit_v(st):
                psf = fil.tile([128, 512], F32, name="psf", tag="fps")
                ps = psf[:, 0:256]
                for kt in range(DKT):
                    nc.tensor.matmul(
                        ps,
                        xslice(kt, 128 * st, 128 * st + 128),
                        wv_t[kt][:],
                        start=(kt == 0),
                        stop=(kt == DKT - 1),
                    )
                dst = vx[st][:].rearrange("p (h c) -> p h c", c=65)[:, :, 0:64]
                nc.vector.tensor_copy(dst, ps.rearrange("p (h c) -> p h c", c=64))

            # filler: strip ss=qs+1 and v tiles 4qs+4..4qs+7 are consumed
            # during strip qs (2 units after each of the 4 head iterations)
            fillers = deque()
            for st in range(4, 16):
                fillers.append(lambda s=st: emit_v(s))
            for ss in range(1, 4):
                for j in range(4):
                    emit_qk_strip(j, ss)


            for qs in range(QS):
                for h in range(HPC):
                    hh = 64 * (h % 2)
                    qT = qks[h // 2][qs][hh : hh + 64, :]

                    # block list: (ki, width, q-col offset within strip)
                    blocks = [(ki, 512, 0) for ki in range(4 * qs)]
                    blocks += [
                        (4 * qs + st, 512 - 128 * st, 128 * st) for st in range(4)
                    ]
                    chunks = [blocks[j : j + 2] for j in range(0, len(blocks), 2)]

                    pts = []
                    for chunk in chunks:
                        w = sum(c[1] for c in chunk)
                        pst = ps_st.tile([128, 1024], F32, tag="pst")
                        off = 0
                        offs = []
                        for ki, cw, qo in chunk:
                            kk = qks[2 + h // 2][ki // 4]
                            nc.tensor.matmul(
                                pst[:, off : off + cw],
                                kk[hh : hh + 64, 128 * (ki % 4) : 128 * (ki % 4) + 128],
                                qT[:, qo : qo + cw],
                                start=True,
                                stop=True,
                            )
                            offs.append(off)
                            off += cw
                        pt = ptp.tile([128, 1024], F16, tag="pt")
                        nc.scalar.activation(
                            pt[:, :w], pst[:, :w], mybir.ActivationFunctionType.Exp
                        )
                        # triangle mask on the first 128 cols of diagonal blocks
                        for (ki, cw, qo), off in zip(chunk, offs):
                            if ki >= 4 * qs:
                                nc.vector.tensor_mul(
                                    pt[:, off : off + 128],
                                    pt[:, off : off + 128],
                                    tri[:],
                                )
                        pts.append((chunk, offs, pt))

                    po = ps_o.tile([65, 512], F32, tag="po")
                    nmm = sum(len(c) for c, _, _ in pts)
                    done = 0
                    for chunk, offs, pt in pts:
                        for (ki, cw, qo), off in zip(chunk, offs):
                            nc.tensor.matmul(
                                po[:, qo : qo + cw],
                                vx[ki][:, 65 * h : 65 * h + 65],
                                pt[:, off : off + cw],
                                start=(done == 0),
                                stop=(done == nmm - 1),
                            )
                            done += 1

                    # normalize: row 64 of po is the softmax denominator
                    rden = nrm.tile([1, 512], F32, tag="rden")
                    nc.vector.tensor_copy(rden[:], po[64:65, :])
                    rrec = nrm.tile([1, 512], F32, tag="rrec")
                    nc.vector.reciprocal_approx_fast(rrec[:], rden[:])
                    rb = nrm.tile([64, 512], F32, tag="rb")
                    nc.gpsimd.partition_broadcast(rb[:], rrec[:])
                    nc.vector.tensor_mul(
                        ots[h // 2][qs][hh : hh + 64, :],
                        po[0:64, :],
                        rb[:],
                    )

                    if fillers:
                        fillers.popleft()()

                # output projection for this strip's 4 token tiles
                for st4 in range(4):
                    for oc in range(2):
                        pop = ups.tile([128, 512], F32, name="pop", tag="upsqk")
                        for t in range(2):
                            nc.tensor.matmul(
                                pop[:],
                                ots[t][qs][:, 128 * st4 : 128 * st4 + 128],
                                wo_t[t][:, 512 * oc : 512 * oc + 512],
                                start=(t == 0),
                                stop=(t == 1),
                            )
                        ob = osb.tile([128, 512], F16, tag="ob")
                        nc.vector.tensor_copy(ob[:], pop[:])
                        nc.sync.dma_start(
                            out[
                                128 * (4 * qs + st4) : 128 * (4 * qs + st4) + 128,
                                512 * oc : 512 * oc + 512,
                            ],
                            ob[:],
                        )

    nc.finalize()
    return nc


_cache = {}


def _get_nc():
    if "causal" not in _cache:
        _cache["causal"] = _build_causal()
    return _cache["causal"]


def _check_causal(mask):
    tril = np.tril(np.ones((S, S), dtype=mask.dtype))
    return all(np.array_equal(np.asarray(mask[b]), tril) for b in range(B))


def _prepare(x, mask, w_qkv, w_out):
    """Host-side sharding. Returns in_maps (one per core)."""
    scale = 1.0 / np.sqrt(DH)

    tri = np.triu(np.ones((128, 128), np.float16))  # keep iff qcol >= krow

    in_maps = []
    for c in range(NCORES):
        b, g = c // 4, c % 4
        heads = range(4 * g, 4 * g + 4)
        xTb = _to_f16(np.ascontiguousarray(x[b].T))
        wq = np.concatenate(
            [w_qkv[:, 64 * h : 64 * h + 64] for h in heads], axis=1
        ) * scale
        wk = np.concatenate(
            [w_qkv[:, D + 64 * h : D + 64 * h + 64] for h in heads], axis=1
        )
        wvv = np.concatenate(
            [w_qkv[:, 2 * D + 64 * h : 2 * D + 64 * h + 64] for h in heads], axis=1
        )
        woo = np.concatenate(
            [w_out[64 * h : 64 * h + 64, :] for h in heads], axis=0
        )
        in_maps.append(
            {
                "xT0": np.ascontiguousarray(xTb[:, 0:512]),
                "xTr": np.ascontiguousarray(xTb[:, 512:]),
                "wqk": _to_f16(np.concatenate([wq, wk], axis=1)),
                "wv": _to_f16(wvv),
                "wo": _to_f16(np.ascontiguousarray(woo)),
                "trimask": tri,
                "vones": np.ones((128, KT * HPC), np.float16),
            }
        )
    return in_maps


def _run(x, mask, w_qkv, w_out, trace=False, trace_cores=None):
    assert _check_causal(mask), "kernel specialized for the causal (tril) mask"
    in_maps = _prepare(x, mask, w_qkv, w_out)
    nc = _get_nc()
    res = run_bass_kernel_spmd(
        nc,
        in_maps,
        core_ids=list(range(NCORES)),
        trace=trace,
        trace_cores=trace_cores,
    )
    outs = np.stack(
        [
            sum(
                res.results[4 * b + g]["out"].astype(np.float32) for g in range(4)
            )
            for b in range(B)
        ]
    )
    return outs.astype(np.float32), res


def kernel(x, mask, w_qkv, w_out):
    x = np.asarray(x, np.float32)
    mask = np.asarray(mask)
    w_qkv = np.asarray(w_qkv, np.float32)
    w_out = np.asarray(w_out, np.float32)
    out, _ = _run(x, mask, w_qkv, w_out)
    return out
